# revision 17
# baseline (speedup 1.0000x reference)
"""Trainium2 Bass kernel for the CTRF dense_cnn problem.

y[b,t,o] = b[o] + sum_{lag in [-10,40]} sum_d W[o,(lag+10)*64+d] * x[b,t-lag,d]

Strategy (8 NeuronCores, data-parallel over batch, 2 batches/core), using a
Winograd F(4,4) decomposition of the 51-tap time conv:

  - 51 taps -> 13 groups of 4 taps (last taps zero-padded). Output tiles of
    4 timesteps (NT = 512 tiles, exactly T). Each (group, tile) contribution
    is F(4,4): 7 transform points instead of 16 tap-applications.
  - Data transform B^T and weight transform G are applied on the HOST; the
    device sees 7 pre-transformed sequences v_p and packed weights.
  - Adjacent groups (2k, 2k+1) are paired on the K dim: stationary
    [128, 128] = [U_{2k,p}; U_{2k+1,p}], moving zV_p = [v_p[c]; v_p[c-1]].
    7 pair-matmuls x 7 points accumulate M_p per 256-col chunk; the A^T
    combine + bias run as progressive scalar_tensor_tensor chains so only
    one DVE pass trails the final matmul.
  - Mixed per-point dtype: points 0..2 bf16, points 3..6 float32r
    (higher node powers amplify quantization error; f32r's ~10-bit
    mantissa keeps total rel err ~6e-3 « 2e-2 while bf16 halves DMA
    for the benign points).
  - PE cols per core: 2 * 7 * 7 * 512 = 50,176 vs 106,496 direct (2.12x).
"""

import os
import sys

os.environ.setdefault("MYCRO_LOCAL_CACHE", "1")

for _p in ("/opt/trn_rl_repo", "/root/.axon_site/_ro/trn_rl_repo"):
    if os.path.isdir(_p) and _p not in sys.path:
        sys.path.insert(0, _p)

import ml_dtypes
import numpy as np

import concourse.mybir as mybir  # noqa: E402
import concourse.tile as tile  # noqa: E402
from concourse import bacc  # noqa: E402
from concourse.bass_utils import run_bass_kernel_spmd  # noqa: E402

N_CORES = 8
B, T, D, O = 16, 2048, 64, 128
NLAGS = 51
BPC = B // N_CORES  # batches per core
M_TILE = 4          # outputs per tile
NPTS = 7            # transform points
NT = T // M_TILE    # 512 output tiles per batch
NG = 13             # tap groups of 4 (taps 51 zero-padded)
NPAIR = 7           # group pairs on K (pair 6 bottom half zero)
C0 = 13             # col offset: c = tau - g + C0
XBASE = 11 - M_TILE - M_TILE * C0  # window origin: x[4c + XBASE + s]
ZC = 526            # zV cols (cc 1..524 used)
ZSPLIT = 288        # head/tail split (chunk0 reads cols < 270)
CW = 256            # chunk width (PSUM: 7 points x half bank, x2 chunks)
N_WARM = 6          # f32 warm matmuls to open the HAM clock gate

# per-point dtype: low-power nodes tolerate bf16; high-power need f32r
PT_BF16 = (True, True, True, False, False, False, False)

# ---- F(4,4) transform matrices, nodes [0, 1, -1, 2, -2, 1/2] + inf -------
_nodes = [0.0, 1.0, -1.0, 2.0, -2.0, 0.5]
_E = np.zeros((7, 4))
for _i, _a in enumerate(_nodes):
    _E[_i] = [_a**_k for _k in range(4)]
_E[6, 3] = 1.0
G_MAT = _E  # filter degree 3 -> same evaluation matrix
_V = np.zeros((7, 7))
for _i, _a in enumerate(_nodes):
    _V[_i] = [_a**_k for _k in range(7)]
_V[6, 6] = 1.0
BT_MAT = np.linalg.inv(_V).T
# A^T = E^T:
#   y0 = M0+M1+M2+M3+M4+M5
#   y1 = M1-M2+2M3-2M4+.5M5
#   y2 = M1+M2+4M3+4M4+.25M5
#   y3 = M1-M2+8M3-8M4+.125M5+M6


def _build():
    nc = bacc.Bacc(
        "TRN2", target_bir_lowering=False, debug=False, num_devices=N_CORES
    )
    f32 = mybir.dt.float32
    bf16 = mybir.dt.bfloat16
    f32r = mybir.dt.float32r
    Alu = mybir.AluOpType
    pt_dt = [bf16 if b_ else f32r for b_ in PT_BF16]

    zv_ds = [
        nc.declare_dram_parameter(f"zv{p}", [BPC, 128, ZC], pt_dt[p], isOutput=False)
        for p in range(NPTS)
    ]
    wg_ds = [
        nc.declare_dram_parameter(f"wg{p}", [128, NPAIR, O], pt_dt[p], isOutput=False)
        for p in range(NPTS)
    ]
    b_d = nc.declare_dram_parameter("bvec", [O, 1], f32, isOutput=False)
    y_d = nc.declare_dram_parameter("y", [BPC, M_TILE, O, NT], f32, isOutput=True)

    with tile.TileContext(nc) as tc:
        with (
            tc.tile_pool(name="consts", bufs=1) as consts,
            tc.tile_pool(name="zv", bufs=1) as zv_pool,
            tc.tile_pool(name="csb", bufs=2) as csb_pool,
            tc.tile_pool(name="ysb", bufs=2) as ysb_pool,
            tc.tile_pool(name="pacc", bufs=8, space="PSUM") as pacc_pool,
        ):
            # HAM warmup (PE clock gate opens after a few us of activity).
            wsrc = consts.tile([128, 128], f32, tag="wsrc")
            nc.vector.memset(wsrc[:], 1.0)
            warm_ps = pacc_pool.tile([128, CW], f32, tag="pacc")
            for _ in range(N_WARM):
                nc.tensor.matmul(
                    warm_ps[:, 0:128], wsrc[:], wsrc[:], start=True, stop=True
                )

            # Input DMAs in consumption order.
            bias_sb = consts.tile([O, 1], f32)
            wg_sbs = []
            zv_sbs = [[None] * NPTS for _ in range(BPC)]
            for p in range(NPTS):
                wg_sb = consts.tile([128, NPAIR, O], pt_dt[p], tag=f"wg{p}")
                nc.sync.dma_start(wg_sb[:], wg_ds[p][:])
                wg_sbs.append(wg_sb)
                zt = zv_pool.tile([128, ZC], pt_dt[p], tag=f"zv0_{p}")
                zv_sbs[0][p] = zt
                nc.sync.dma_start(zt[:, 0:ZSPLIT], zv_ds[p][0, :, 0:ZSPLIT])
                if p == 0:
                    nc.sync.dma_start(bias_sb[:], b_d[:])
            for p in range(NPTS):
                nc.sync.dma_start(
                    zv_sbs[0][p][:, ZSPLIT:], zv_ds[p][0, :, ZSPLIT:]
                )
            for bb in range(1, BPC):
                for p in range(NPTS):
                    zt = zv_pool.tile([128, ZC], pt_dt[p], tag=f"zv{bb}_{p}")
                    zv_sbs[bb][p] = zt
                    nc.sync.dma_start(zt[:, 0:ZSPLIT], zv_ds[p][bb, :, 0:ZSPLIT])
                for p in range(NPTS):
                    nc.sync.dma_start(
                        zv_sbs[bb][p][:, ZSPLIT:], zv_ds[p][bb, :, ZSPLIT:]
                    )

            def ctile(tag):
                return csb_pool.tile([128, CW], f32, tag=tag, name=tag)

            def stt(out, psum, scalar, sbuf, op0):
                nc.vector.scalar_tensor_tensor(
                    out[:, 0:CW], psum[:, 0:CW], scalar, sbuf[:, 0:CW],
                    op0, Alu.add,
                )

            def emit_chunk(bb, t0):
                M = []
                for p in range(NPTS):
                    pacc = pacc_pool.tile([128, CW], f32, tag="pacc")
                    M.append(pacc)
                    for k in range(NPAIR):
                        off = C0 - 2 * k
                        nc.tensor.matmul(
                            pacc[:],
                            wg_sbs[p][:, k, :],
                            zv_sbs[bb][p][:, t0 + off : t0 + off + CW],
                            start=(k == 0),
                            stop=(k == NPAIR - 1),
                        )
                    # progressive A^T combine (DVE reads <=1 PSUM per op)
                    if p == 0:
                        q = ctile("q")
                        nc.vector.tensor_scalar_add(
                            q[:, 0:CW], M[0][:, 0:CW], bias_sb[:]
                        )
                    if p == 1:
                        s1b = ctile("s1b")
                        nc.vector.tensor_scalar_add(
                            s1b[:, 0:CW], M[1][:, 0:CW], bias_sb[:]
                        )
                        q2 = ctile("q")
                        stt(q2, M[1], 0.0, q, Alu.add)
                        q = q2
                    if p == 2:
                        dd = ctile("d")
                        stt(dd, M[2], -1.0, s1b, Alu.mult)
                        t2 = ctile("t2")
                        stt(t2, M[2], 0.0, s1b, Alu.add)
                        q2 = ctile("q")
                        stt(q2, M[2], 0.0, q, Alu.add)
                        q = q2
                    if p == 3:
                        q2 = ctile("q")
                        stt(q2, M[3], 0.0, q, Alu.add)
                        q = q2
                        u = ctile("u")
                        stt(u, M[3], 2.0, dd, Alu.mult)
                        v = ctile("v")
                        stt(v, M[3], 4.0, t2, Alu.mult)
                        w = ctile("w")
                        stt(w, M[3], 8.0, dd, Alu.mult)
                    if p == 4:
                        q2 = ctile("q")
                        stt(q2, M[4], 0.0, q, Alu.add)
                        q = q2
                        u2 = ctile("u")
                        stt(u2, M[4], -2.0, u, Alu.mult)
                        u = u2
                        v2 = ctile("v")
                        stt(v2, M[4], 4.0, v, Alu.mult)
                        v = v2
                        w2 = ctile("w")
                        stt(w2, M[4], -8.0, w, Alu.mult)
                        w = w2
                    if p == 5:
                        y0 = ysb_pool.tile([128, CW], f32, tag="y0")
                        stt(y0, M[5], 0.0, q, Alu.add)
                        nc.sync.dma_start(y_d[bb, 0, :, t0 : t0 + CW], y0[:])
                        y1 = ysb_pool.tile([128, CW], f32, tag="y1")
                        stt(y1, M[5], 0.5, u, Alu.mult)
                        nc.sync.dma_start(y_d[bb, 1, :, t0 : t0 + CW], y1[:])
                        y2 = ysb_pool.tile([128, CW], f32, tag="y2")
                        stt(y2, M[5], 0.25, v, Alu.mult)
                        nc.sync.dma_start(y_d[bb, 2, :, t0 : t0 + CW], y2[:])
                        w3 = ctile("w")
                        stt(w3, M[5], 0.125, w, Alu.mult)
                        w = w3
                # y3 = M6 + w — the only pass after the final matmul
                y3 = ysb_pool.tile([128, CW], f32, tag="y3")
                stt(y3, M[6], 0.0, w, Alu.add)
                nc.sync.dma_start(y_d[bb, 3, :, t0 : t0 + CW], y3[:])

            for bb in range(BPC):
                for t0 in range(0, NT, CW):
                    emit_chunk(bb, t0)
    nc.compile()
    return nc


_NC_CACHE = {}


def _get_program():
    if "nc" not in _NC_CACHE:
        _NC_CACHE["nc"] = _build()
    return _NC_CACHE["nc"]


def _prep_inputs(x, W, b):
    x = np.ascontiguousarray(x, dtype=np.float32)
    W = np.ascontiguousarray(W, dtype=np.float32)
    b = np.ascontiguousarray(b, dtype=np.float32)

    # --- data transform: v_p[c] = sum_s BT[p,s] x[4c + XBASE + s] ---------
    PAD = 80
    xpad = np.zeros((B, T + 2 * PAD, D), dtype=np.float32)
    xpad[:, PAD : PAD + T] = x
    cs = np.arange(-1, ZC)  # c for top cols 0..ZC-1, bottom needs c-1 >= -1
    idx = PAD + M_TILE * cs[:, None] + XBASE + np.arange(NPTS)[None, :]
    dwin = xpad[:, idx]                          # [B, nc, 7, D]
    v = np.einsum(
        "ps,bcsd->bpcd", BT_MAT.astype(np.float32), dwin
    )                                            # [B, 7, nc, D]
    vt = v.transpose(0, 1, 3, 2)                 # [B, 7, D, nc]; col k = c+1
    zv_f32 = np.zeros((B, NPTS, 128, ZC), dtype=np.float32)
    zv_f32[:, :, :D, :] = vt[:, :, :, 1 : 1 + ZC]
    zv_f32[:, :, D:, :] = vt[:, :, :, 0:ZC]

    # --- weight transform -------------------------------------------------
    # U_{g,p}[d, o] = sum_i G[p,i] * W[o, (4g+3-i)*64 + d], tap >= 51 -> 0
    Wblk = W.reshape(O, NLAGS, D)
    wg = np.zeros((NPTS, 128, NPAIR, O), dtype=np.float32)
    for p in range(NPTS):
        for g in range(NG):
            U = np.zeros((D, O), dtype=np.float32)
            for i in range(M_TILE):
                tap = M_TILE * g + M_TILE - 1 - i
                if tap < NLAGS:
                    U += np.float32(G_MAT[p, i]) * Wblk[:, tap, :].T
            k, half = divmod(g, 2)
            wg[p, half * D : (half + 1) * D, k, :] = U

    def cast(a, p):
        a = np.ascontiguousarray(a)
        return a.astype(ml_dtypes.bfloat16) if PT_BF16[p] else a

    wg_maps = {f"wg{p}": cast(wg[p], p) for p in range(NPTS)}
    bvec = np.ascontiguousarray(b.reshape(O, 1))
    maps = []
    for c in range(N_CORES):
        m = {"bvec": bvec}
        for p in range(NPTS):
            m[f"zv{p}"] = cast(zv_f32[c * BPC : (c + 1) * BPC, p], p)
        m.update(wg_maps)
        maps.append(m)
    return maps


def _assemble(res):
    # Per core: y_raw [BPC, 4, O, NT]; y[b, 4*tau+r, o] = y_raw[b, r, o, tau]
    outs = []
    for c in range(N_CORES):
        y_raw = res.results[c]["y"]
        y = (
            y_raw.transpose(0, 3, 1, 2)
            .reshape(BPC, T, O)
            .astype(np.float32)
        )
        outs.append(np.ascontiguousarray(y))
    return np.concatenate(outs, axis=0)


def kernel(x, W, b):
    in_maps = _prep_inputs(x, W, b)
    res = run_bass_kernel_spmd(
        _get_program(), in_maps, core_ids=list(range(N_CORES))
    )
    return _assemble(res)


def _ensure_ntff_hook():
    """The agent image's antenv lacks axon_hooks, so run_bass_kernel_spmd's
    trace path degrades to no-profile. Seed an equivalent module backed by
    the ctypes NTFF profiler from trn_agent_boot."""
    try:
        from antenv.axon_hooks import get_axon_ntff_profile_hook

        if get_axon_ntff_profile_hook() is not None:
            return True
    except ImportError:
        pass
    try:
        import types

        site_dir = "/root/.axon_site"
        if site_dir not in sys.path and os.path.isdir(site_dir):
            sys.path.insert(0, site_dir)
        from trn_agent_boot.trn_boot import _ntff_profile_via_ctypes

        hook = _ntff_profile_via_ctypes("/opt/axon/libaxon_pjrt.so")
        if hook is None:
            return False
        mod = types.ModuleType("antenv.axon_hooks")
        mod.get_axon_ntff_profile_hook = lambda: hook
        mod.set_axon_ntff_profile_hook = lambda h: None
        sys.modules["antenv.axon_hooks"] = mod
        import antenv

        antenv.axon_hooks = mod
        return True
    except Exception:
        return False


def kernel_traced(x, W, b, **kwargs):
    """Like kernel() but requests an NTFF trace; returns (y, BassKernelResults).

    Dev-loop only (test.py); the graded kernel() path never traces. The
    artifact upload is stubbed out since this container has no bucket access.
    """
    _ensure_ntff_hook()
    from concourse import bass_utils as _bu

    in_maps = _prep_inputs(x, W, b)
    orig_upload = _bu.upload_artifacts
    _bu.upload_artifacts = lambda tmpdir: f"local:{tmpdir}"
    try:
        res = run_bass_kernel_spmd(
            _get_program(), in_maps, core_ids=list(range(N_CORES)), trace=True, **kwargs
        )
    finally:
        _bu.upload_artifacts = orig_upload
    y = _assemble(res)
    return y, res


# revision 18
# speedup vs baseline: 1.2152x; 1.2152x over previous
"""Trainium2 Bass kernel for the CTRF dense_cnn problem.

y[b,t,o] = b[o] + sum_{lag in [-10,40]} sum_d W[o,(lag+10)*64+d] * x[b,t-lag,d]

Strategy (8 NeuronCores, data-parallel over batch, 2 batches/core), using a
Winograd F(4,4) decomposition of the 51-tap time conv:

  - 51 taps -> 13 groups of 4 taps (last taps zero-padded). Output tiles of
    4 timesteps (NT = 512 tiles, exactly T). Each (group, tile) contribution
    is F(4,4): 7 transform points instead of 16 tap-applications.
  - Data transform B^T and weight transform G are applied on the HOST; the
    device sees 7 pre-transformed sequences v_p and packed weights.
  - Adjacent groups (2k, 2k+1) are paired on the K dim: stationary
    [128, 128] = [U_{2k,p}; U_{2k+1,p}], moving zV_p = [v_p[c]; v_p[c-1]].
    7 pair-matmuls x 7 points accumulate M_p per 256-col chunk; the A^T
    combine + bias run as progressive scalar_tensor_tensor chains so only
    one DVE pass trails the final matmul.
  - Mixed per-point dtype: points 0..2 bf16, points 3..6 float32r
    (higher node powers amplify quantization error; f32r's ~10-bit
    mantissa keeps total rel err ~6e-3 « 2e-2 while bf16 halves DMA
    for the benign points).
  - PE cols per core: 2 * 7 * 7 * 512 = 50,176 vs 106,496 direct (2.12x).
"""

import os
import sys

os.environ.setdefault("MYCRO_LOCAL_CACHE", "1")

for _p in ("/opt/trn_rl_repo", "/root/.axon_site/_ro/trn_rl_repo"):
    if os.path.isdir(_p) and _p not in sys.path:
        sys.path.insert(0, _p)

import ml_dtypes
import numpy as np

import concourse.mybir as mybir  # noqa: E402
import concourse.tile as tile  # noqa: E402
from concourse import bacc  # noqa: E402
from concourse.bass_utils import run_bass_kernel_spmd  # noqa: E402

N_CORES = 8
B, T, D, O = 16, 2048, 64, 128
NLAGS = 51
BPC = B // N_CORES  # batches per core
M_TILE = 4          # outputs per tile
NPTS = 7            # transform points
NT = T // M_TILE    # 512 output tiles per batch
NG = 13             # tap groups of 4 (taps 51 zero-padded)
NPAIR = 7           # group pairs on K (pair 6 bottom half zero)
C0 = 13             # col offset: c = tau - g + C0
XBASE = 11 - M_TILE - M_TILE * C0  # window origin: x[4c + XBASE + s]
ZC = 526            # zV cols (cc 1..524 used)
CW = 512            # chunk width: one chunk per batch (7 banks + warm)
# matmul/point order: bf16 points first to build DMA slack for f32r points
POINT_ORDER = (0, 1, 2, 5, 3, 4, 6)
N_WARM = 6          # f32 warm matmuls to open the HAM clock gate

# per-point dtype: low-power nodes tolerate bf16; high-power need f32r
PT_BF16 = (True, True, True, False, False, False, False)

# ---- F(4,4) transform matrices, nodes [0, 1, -1, 2, -2, 1/2] + inf -------
_nodes = [0.0, 1.0, -1.0, 2.0, -2.0, 0.5]
_E = np.zeros((7, 4))
for _i, _a in enumerate(_nodes):
    _E[_i] = [_a**_k for _k in range(4)]
_E[6, 3] = 1.0
G_MAT = _E  # filter degree 3 -> same evaluation matrix
_V = np.zeros((7, 7))
for _i, _a in enumerate(_nodes):
    _V[_i] = [_a**_k for _k in range(7)]
_V[6, 6] = 1.0
BT_MAT = np.linalg.inv(_V).T
# A^T = E^T:
#   y0 = M0+M1+M2+M3+M4+M5
#   y1 = M1-M2+2M3-2M4+.5M5
#   y2 = M1+M2+4M3+4M4+.25M5
#   y3 = M1-M2+8M3-8M4+.125M5+M6


def _build():
    nc = bacc.Bacc(
        "TRN2", target_bir_lowering=False, debug=False, num_devices=N_CORES
    )
    f32 = mybir.dt.float32
    bf16 = mybir.dt.bfloat16
    f32r = mybir.dt.float32r
    Alu = mybir.AluOpType
    pt_dt = [bf16 if b_ else f32r for b_ in PT_BF16]

    zv_ds = [
        nc.declare_dram_parameter(f"zv{p}", [BPC, 128, ZC], pt_dt[p], isOutput=False)
        for p in range(NPTS)
    ]
    wg_ds = [
        nc.declare_dram_parameter(f"wg{p}", [128, NPAIR, O], pt_dt[p], isOutput=False)
        for p in range(NPTS)
    ]
    b_d = nc.declare_dram_parameter("bvec", [O, 1], f32, isOutput=False)
    y_d = nc.declare_dram_parameter("y", [BPC, M_TILE, O, NT], f32, isOutput=True)

    with tile.TileContext(nc) as tc:
        with (
            tc.tile_pool(name="consts", bufs=1) as consts,
            tc.tile_pool(name="zv", bufs=1) as zv_pool,
            tc.tile_pool(name="csb", bufs=2) as csb_pool,
            tc.tile_pool(name="ysb", bufs=2) as ysb_pool,
            tc.tile_pool(name="pacc", bufs=8, space="PSUM") as pacc_pool,
        ):
            # HAM warmup (PE clock gate opens after a few us of activity).
            wsrc = consts.tile([128, 128], f32, tag="wsrc")
            nc.vector.memset(wsrc[:], 1.0)
            warm_ps = pacc_pool.tile([128, CW], f32, tag="pacc")
            for _ in range(N_WARM):
                nc.tensor.matmul(
                    warm_ps[:, 0:128], wsrc[:], wsrc[:], start=True, stop=True
                )

            # Input DMAs in consumption order (POINT_ORDER, batch-major).
            bias_sb = consts.tile([O, 1], f32)
            wg_sbs = [None] * NPTS
            zv_sbs = [[None] * NPTS for _ in range(BPC)]
            first = True
            for p in POINT_ORDER:
                wg_sb = consts.tile([128, NPAIR, O], pt_dt[p], tag=f"wg{p}")
                nc.sync.dma_start(wg_sb[:], wg_ds[p][:])
                wg_sbs[p] = wg_sb
                zt = zv_pool.tile([128, ZC], pt_dt[p], tag=f"zv0_{p}")
                zv_sbs[0][p] = zt
                nc.sync.dma_start(zt[:], zv_ds[p][0])
                if first:
                    nc.sync.dma_start(bias_sb[:], b_d[:])
                    first = False
            for bb in range(1, BPC):
                for p in POINT_ORDER:
                    zt = zv_pool.tile([128, ZC], pt_dt[p], tag=f"zv{bb}_{p}")
                    zv_sbs[bb][p] = zt
                    nc.sync.dma_start(zt[:], zv_ds[p][bb])

            def ctile(tag):
                return csb_pool.tile([128, CW], f32, tag=tag, name=tag)

            def stt(out, psum, scalar, sbuf, op0):
                nc.vector.scalar_tensor_tensor(
                    out[:, 0:CW], psum[:, 0:CW], scalar, sbuf[:, 0:CW],
                    op0, Alu.add,
                )

            def emit_chunk(bb, t0):
                M = {}
                q = s1b = dd = t2 = d5 = t5 = w5 = u = v = w = None
                for p in POINT_ORDER:
                    pacc = pacc_pool.tile([128, CW], f32, tag="pacc")
                    M[p] = pacc
                    for k in range(NPAIR):
                        off = C0 - 2 * k
                        nc.tensor.matmul(
                            pacc[:],
                            wg_sbs[p][:, k, :],
                            zv_sbs[bb][p][:, t0 + off : t0 + off + CW],
                            start=(k == 0),
                            stop=(k == NPAIR - 1),
                        )
                    # progressive A^T combine (DVE reads <=1 PSUM per op)
                    if p == 0:
                        q = ctile("q")
                        nc.vector.tensor_scalar_add(
                            q[:, 0:CW], M[0][:, 0:CW], bias_sb[:]
                        )
                    elif p == 1:
                        s1b = ctile("s1b")
                        nc.vector.tensor_scalar_add(
                            s1b[:, 0:CW], M[1][:, 0:CW], bias_sb[:]
                        )
                        q2 = ctile("q")
                        stt(q2, M[1], 0.0, q, Alu.add)
                        q = q2
                    elif p == 2:
                        dd = ctile("d")
                        stt(dd, M[2], -1.0, s1b, Alu.mult)
                        t2 = ctile("t2")
                        stt(t2, M[2], 0.0, s1b, Alu.add)
                        q2 = ctile("q")
                        stt(q2, M[2], 0.0, q, Alu.add)
                        q = q2
                    elif p == 5:
                        q2 = ctile("q")
                        stt(q2, M[5], 0.0, q, Alu.add)
                        q = q2
                        d5 = ctile("d5")
                        stt(d5, M[5], 0.5, dd, Alu.mult)
                        t5 = ctile("t5")
                        stt(t5, M[5], 0.25, t2, Alu.mult)
                        w5 = ctile("w5")
                        stt(w5, M[5], 0.125, dd, Alu.mult)
                    elif p == 3:
                        q2 = ctile("q")
                        stt(q2, M[3], 0.0, q, Alu.add)
                        q = q2
                        u = ctile("u")
                        stt(u, M[3], 2.0, d5, Alu.mult)
                        v = ctile("v")
                        stt(v, M[3], 4.0, t5, Alu.mult)
                        w = ctile("w")
                        stt(w, M[3], 8.0, w5, Alu.mult)
                    elif p == 4:
                        y0 = ysb_pool.tile([128, CW], f32, tag="y0")
                        stt(y0, M[4], 0.0, q, Alu.add)
                        nc.sync.dma_start(y_d[bb, 0, :, t0 : t0 + CW], y0[:])
                        y1 = ysb_pool.tile([128, CW], f32, tag="y1")
                        stt(y1, M[4], -2.0, u, Alu.mult)
                        nc.sync.dma_start(y_d[bb, 1, :, t0 : t0 + CW], y1[:])
                        y2 = ysb_pool.tile([128, CW], f32, tag="y2")
                        stt(y2, M[4], 4.0, v, Alu.mult)
                        nc.sync.dma_start(y_d[bb, 2, :, t0 : t0 + CW], y2[:])
                        w2 = ctile("w")
                        stt(w2, M[4], -8.0, w, Alu.mult)
                        w = w2
                # y3 = M6 + w — the only pass after the final matmul
                y3 = ysb_pool.tile([128, CW], f32, tag="y3")
                stt(y3, M[6], 0.0, w, Alu.add)
                nc.sync.dma_start(y_d[bb, 3, :, t0 : t0 + CW], y3[:])

            for bb in range(BPC):
                for t0 in range(0, NT, CW):
                    emit_chunk(bb, t0)
    nc.compile()
    return nc


_NC_CACHE = {}


def _get_program():
    if "nc" not in _NC_CACHE:
        _NC_CACHE["nc"] = _build()
    return _NC_CACHE["nc"]


def _prep_inputs(x, W, b):
    x = np.ascontiguousarray(x, dtype=np.float32)
    W = np.ascontiguousarray(W, dtype=np.float32)
    b = np.ascontiguousarray(b, dtype=np.float32)

    # --- data transform: v_p[c] = sum_s BT[p,s] x[4c + XBASE + s] ---------
    PAD = 80
    xpad = np.zeros((B, T + 2 * PAD, D), dtype=np.float32)
    xpad[:, PAD : PAD + T] = x
    cs = np.arange(-1, ZC)  # c for top cols 0..ZC-1, bottom needs c-1 >= -1
    idx = PAD + M_TILE * cs[:, None] + XBASE + np.arange(NPTS)[None, :]
    dwin = xpad[:, idx]                          # [B, nc, 7, D]
    v = np.einsum(
        "ps,bcsd->bpcd", BT_MAT.astype(np.float32), dwin
    )                                            # [B, 7, nc, D]
    vt = v.transpose(0, 1, 3, 2)                 # [B, 7, D, nc]; col k = c+1
    zv_f32 = np.zeros((B, NPTS, 128, ZC), dtype=np.float32)
    zv_f32[:, :, :D, :] = vt[:, :, :, 1 : 1 + ZC]
    zv_f32[:, :, D:, :] = vt[:, :, :, 0:ZC]

    # --- weight transform -------------------------------------------------
    # U_{g,p}[d, o] = sum_i G[p,i] * W[o, (4g+3-i)*64 + d], tap >= 51 -> 0
    Wblk = W.reshape(O, NLAGS, D)
    wg = np.zeros((NPTS, 128, NPAIR, O), dtype=np.float32)
    for p in range(NPTS):
        for g in range(NG):
            U = np.zeros((D, O), dtype=np.float32)
            for i in range(M_TILE):
                tap = M_TILE * g + M_TILE - 1 - i
                if tap < NLAGS:
                    U += np.float32(G_MAT[p, i]) * Wblk[:, tap, :].T
            k, half = divmod(g, 2)
            wg[p, half * D : (half + 1) * D, k, :] = U

    def cast(a, p):
        a = np.ascontiguousarray(a)
        return a.astype(ml_dtypes.bfloat16) if PT_BF16[p] else a

    wg_maps = {f"wg{p}": cast(wg[p], p) for p in range(NPTS)}
    bvec = np.ascontiguousarray(b.reshape(O, 1))
    maps = []
    for c in range(N_CORES):
        m = {"bvec": bvec}
        for p in range(NPTS):
            m[f"zv{p}"] = cast(zv_f32[c * BPC : (c + 1) * BPC, p], p)
        m.update(wg_maps)
        maps.append(m)
    return maps


def _assemble(res):
    # Per core: y_raw [BPC, 4, O, NT]; y[b, 4*tau+r, o] = y_raw[b, r, o, tau]
    outs = []
    for c in range(N_CORES):
        y_raw = res.results[c]["y"]
        y = (
            y_raw.transpose(0, 3, 1, 2)
            .reshape(BPC, T, O)
            .astype(np.float32)
        )
        outs.append(np.ascontiguousarray(y))
    return np.concatenate(outs, axis=0)


def kernel(x, W, b):
    in_maps = _prep_inputs(x, W, b)
    res = run_bass_kernel_spmd(
        _get_program(), in_maps, core_ids=list(range(N_CORES))
    )
    return _assemble(res)


def _ensure_ntff_hook():
    """The agent image's antenv lacks axon_hooks, so run_bass_kernel_spmd's
    trace path degrades to no-profile. Seed an equivalent module backed by
    the ctypes NTFF profiler from trn_agent_boot."""
    try:
        from antenv.axon_hooks import get_axon_ntff_profile_hook

        if get_axon_ntff_profile_hook() is not None:
            return True
    except ImportError:
        pass
    try:
        import types

        site_dir = "/root/.axon_site"
        if site_dir not in sys.path and os.path.isdir(site_dir):
            sys.path.insert(0, site_dir)
        from trn_agent_boot.trn_boot import _ntff_profile_via_ctypes

        hook = _ntff_profile_via_ctypes("/opt/axon/libaxon_pjrt.so")
        if hook is None:
            return False
        mod = types.ModuleType("antenv.axon_hooks")
        mod.get_axon_ntff_profile_hook = lambda: hook
        mod.set_axon_ntff_profile_hook = lambda h: None
        sys.modules["antenv.axon_hooks"] = mod
        import antenv

        antenv.axon_hooks = mod
        return True
    except Exception:
        return False


def kernel_traced(x, W, b, **kwargs):
    """Like kernel() but requests an NTFF trace; returns (y, BassKernelResults).

    Dev-loop only (test.py); the graded kernel() path never traces. The
    artifact upload is stubbed out since this container has no bucket access.
    """
    _ensure_ntff_hook()
    from concourse import bass_utils as _bu

    in_maps = _prep_inputs(x, W, b)
    orig_upload = _bu.upload_artifacts
    _bu.upload_artifacts = lambda tmpdir: f"local:{tmpdir}"
    try:
        res = run_bass_kernel_spmd(
            _get_program(), in_maps, core_ids=list(range(N_CORES)), trace=True, **kwargs
        )
    finally:
        _bu.upload_artifacts = orig_upload
    y = _assemble(res)
    return y, res
